# revision 1
# baseline (speedup 1.0000x reference)
"""BiAttention Trainium2 Bass kernel, v7: i-half-split software pipeline.

Data-parallel over batch: 16 batches -> 8 cores x 2 batches.

The i-range (2048) is split into two halves A/B.  Per batch k:
  P1A(k): cross+exp for i in A (all jt)  -> WT_A
  P1B(k): cross+exp for i in B          -> WT_B
  P2A(k): out_one MMs for i in A using WT_A  -- interleaved into P1B(k)
  P2B(k): out_one MMs for i in B             -- interleaved into P1A(k+1)
so the PE out_one work hides entirely under the ACT exp floor, and WT
needs only 2x32KB with no double buffering.  out_two*out_one (needs
out2b, ready after both halves) is deferred into P1A(k+1) slots.

Engine budget per batch: ACT exp 32us (floor), DVE ~45us (macc+md/id+
fp8 cast copies+m0+stats), POOL ~33us (in*o1, o2*o1, o1*rden, casts),
PE ~40us (fp8 DoubleRow cross + bf16 out_one + transposes).
"""

import os
import sys

for _p in ("/opt/trn_rl_repo", "/root/.axon_site/_ro/trn_rl_repo"):
    if os.path.isdir(_p) and _p not in sys.path:
        sys.path.insert(0, _p)

import numpy as np

import concourse.bass as bass
import concourse.tile as tile
from concourse import bacc, mybir
from concourse.masks import make_identity

B, LI, LM, D = 16, 2048, 2048, 256
N_CORES = 8
B_PC = B // N_CORES

F32 = mybir.dt.float32
BF16 = mybir.dt.bfloat16
FP8 = mybir.dt.float8e4

MS = 8.0
FP8_SCALE = 64.0


def build_program(b_pc=B_PC, li=LI, lm=LM, d=D, reps=1, cross_fp8=False):
    nc = bacc.Bacc("TRN2", target_bir_lowering=False, debug=False)

    inp = nc.declare_dram_parameter("input", [b_pc, li, d], F32, isOutput=False).ap()
    mem = nc.declare_dram_parameter("memory", [b_pc, lm, d], F32, isOutput=False).ap()
    msk = nc.declare_dram_parameter("mask", [b_pc, lm], F32, isOutput=False).ap()
    w_in = nc.declare_dram_parameter("w_input", [d], F32, isOutput=False).ap()
    w_mem = nc.declare_dram_parameter("w_memory", [d], F32, isOutput=False).ap()
    dsc = nc.declare_dram_parameter("dot_scale", [d], F32, isOutput=False).ap()
    out = nc.declare_dram_parameter("output", [b_pc, li, 4 * d], F32, isOutput=True).ap()

    NTI = li // 128       # 16
    NTJ = lm // 128       # 16
    KD = d // 128         # 2
    DE = d + 1
    HI = li // 2          # 1024 (i per half)
    NTH = NTI // 2        # 8 (i-tiles per half)
    XD = FP8 if cross_fp8 else BF16

    AL = mybir.AluOpType
    AF = mybir.ActivationFunctionType

    with tile.TileContext(nc) as tc:
        with (
            tc.tile_pool(name="singles", bufs=1) as singles,
            tc.tile_pool(name="loads", bufs=2) as loads,
            tc.tile_pool(name="wt", bufs=1) as wtp,
            tc.tile_pool(name="work", bufs=2) as work,
            tc.tile_pool(name="stats", bufs=2) as stats,
            tc.tile_pool(name="outp", bufs=3) as outp,
            tc.tile_pool(name="ps_att", bufs=2, space="PSUM") as ps_att,
            tc.tile_pool(name="ps_tp", bufs=2, space="PSUM") as ps_tp,
            tc.tile_pool(name="ps_acc", bufs=2, space="PSUM") as ps_acc,
        ):
            ident_bf = singles.tile([128, 128], BF16, tag="identb")
            make_identity(nc, ident_bf)
            ident_f32 = singles.tile([128, 128], F32, tag="identf")
            make_identity(nc, ident_f32)
            ones_row = singles.tile([1, 128], F32, tag="ones")
            nc.vector.memset(ones_row, 1.0)

            def bcast128(ap):
                return bass.AP(tensor=ap.tensor, offset=ap.offset, ap=[[0, 128]] + list(ap.ap))

            w_in_b = singles.tile([128, d], F32, tag="winb")
            nc.sync.dma_start(out=w_in_b, in_=bcast128(w_in))
            w_mem_b = singles.tile([128, d], F32, tag="wmemb")
            nc.sync.dma_start(out=w_mem_b, in_=bcast128(w_mem))
            ds_t = singles.tile([128, KD], F32, tag="dst")
            nc.sync.dma_start(out=ds_t, in_=dsc.rearrange("(c p) -> p c", p=128))
            ds64T = singles.tile([128, KD], F32, tag="ds64t")
            nc.vector.tensor_scalar(
                out=ds64T, in0=ds_t,
                scalar1=FP8_SCALE, scalar2=None, op0=AL.mult,
            )

            batches = [bb for _ in range(reps) for bb in range(b_pc)]
            K = len(batches)
            st = {}

            # ---------- P0 pieces ----------
            def c_loads(k):
                b = batches[k]
                s = st.setdefault(k, {})
                s["in_sb"] = loads.tile(
                    [128, NTI, d], F32, name="in_sb", tag="in_sb", bufs=3
                )
                s["mem_sb"] = loads.tile([128, NTJ, d], F32, name="mem_sb", tag="mem_sb", bufs=1)
                in_v = inp[b].rearrange("(t p) d -> p t d", p=128)
                mem_v = mem[b].rearrange("(t p) d -> p t d", p=128)
                nc.sync.dma_start(out=s["mem_sb"][:, 0:8, :], in_=mem_v[:, 0:8, :])
                nc.sync.dma_start(out=s["in_sb"][:, 0:8, :], in_=in_v[:, 0:8, :])
                nc.sync.dma_start(out=s["mem_sb"][:, 8:16, :], in_=mem_v[:, 8:16, :])
                nc.sync.dma_start(out=s["in_sb"][:, 8:16, :], in_=in_v[:, 8:16, :])
                s["mask_sb"] = loads.tile([128, NTJ], F32, name="mask_sb", tag="mask_sb")
                nc.sync.dma_start(
                    out=s["mask_sb"], in_=msk[b].rearrange("(t p) -> p t", p=128)
                )

            def c_casts(k, half=None):
                s = st[k]
                lo, hi = (0, NTJ) if half is None else (8 * half, 8 * half + 8)
                if half in (None, 0):
                    s["mem_ext"] = work.tile([128, NTJ, DE], BF16, name="mem_ext", tag="mem_ext")
                    nc.vector.memset(s["mem_ext"][:, :, d : d + 1], 1.0)
                nc.gpsimd.tensor_copy(
                    s["mem_ext"][:, lo:hi, 0:d], s["mem_sb"][:, lo:hi, :]
                )

            def c_mdid(k, jr, which="both"):
                s = st[k]
                if "scr" not in s:
                    s["scr"] = stats.tile([128, d], F32, name="scr", tag="scr", bufs=1)
                    s["mdT"] = stats.tile([128, NTJ], F32, name="mdT", tag="mdT")
                    s["idT"] = stats.tile([128, NTI], F32, name="idT", tag="idT")
                for q in range(2):
                    jt = jr * 2 + q
                    if which in ("both", "md"):
                        nc.vector.affine_mul_reduce(
                            out=s["scr"], accum_out=s["mdT"][:, jt : jt + 1],
                            in0=s["mem_sb"][:, jt, :], in1=w_mem_b, scale=1.0, bias=0.0,
                        )
                    if which in ("both", "id"):
                        nc.vector.affine_mul_reduce(
                            out=s["scr"], accum_out=s["idT"][:, jt : jt + 1],
                            in0=s["in_sb"][:, jt, :], in1=w_in_b, scale=1.0, bias=0.0,
                        )

            def c_tp_mem(k, j0):
                s = st[k]
                if j0 == 0:
                    s["memT8"] = work.tile([128, KD, lm], XD, name="memT8", tag="memT8")
                pst = ps_tp.tile([128, 4, 128], BF16, tag="tp", bufs=1)
                for c in range(KD):
                    for g in range(2):
                        nc.tensor.transpose(
                            pst[:, c * 2 + g, :],
                            s["mem_ext"][:, j0 + g, c * 128 : (c + 1) * 128],
                            ident_bf,
                        )
                nc.scalar.copy(
                    s["memT8"][:, :, j0 * 128 : (j0 + 2) * 128].rearrange(
                        "p c (g x) -> p c g x", g=2
                    ),
                    pst.rearrange("p (c g) x -> p c g x", g=2),
                )

            def c_tp_in(k, i0):
                s = st[k]
                if i0 == 0:
                    s["insT8"] = work.tile([128, KD, li], XD, name="insT8", tag="insT8")
                pstf = ps_tp.tile([128, 2, 2, 128], F32, tag="tpf", bufs=1)
                for c in range(KD):
                    for g in range(2):
                        nc.tensor.transpose(
                            pstf[:, c, g, :],
                            s["in_sb"][:, i0 + g, c * 128 : (c + 1) * 128],
                            ident_f32,
                        )
                for c in range(KD):
                    nc.vector.tensor_scalar_mul(
                        s["insT8"][:, c, i0 * 128 : (i0 + 2) * 128],
                        pstf[:, c, :, :].rearrange("p g x -> p (g x)"),
                        ds64T[:, c : c + 1],
                    )

            def p0_tail(k):
                s = st[k]
                t_ms = stats.tile([128, NTJ], F32, tag="t_ms")
                nc.vector.tensor_scalar(
                    out=t_ms, in0=s["mask_sb"], scalar1=-1.0, scalar2=1e30,
                    op0=AL.add, op1=AL.mult,
                )
                mdT2 = stats.tile([128, NTJ], F32, tag="mdT2")
                nc.vector.tensor_add(mdT2, s["mdT"], t_ms)
                mm_p = stats.tile([128, 1], F32, tag="mm_p")
                nc.vector.reduce_max(mm_p, mdT2, axis=mybir.AxisListType.X)
                ps_r = ps_tp.tile([1, 128], F32, tag="tp", bufs=1)
                nc.tensor.transpose(ps_r, mm_p, ident_f32)
                mm_s = stats.tile([1, 1], F32, tag="mm_s")
                nc.vector.reduce_max(mm_s, ps_r, axis=mybir.AxisListType.X)
                negmd8 = stats.tile([1, 1], F32, tag="negmd8")
                nc.vector.tensor_scalar(
                    out=negmd8, in0=mm_s, scalar1=-1.0, scalar2=-MS,
                    op0=AL.mult, op1=AL.add,
                )
                ps_b1 = ps_acc.tile([128, 1], F32, tag="acc")
                nc.tensor.matmul(ps_b1, lhsT=ones_row, rhs=negmd8, start=True, stop=True)
                negmd8_b = stats.tile([128, 1], F32, tag="negmd8b")
                nc.vector.tensor_copy(negmd8_b, ps_b1)
                s["mdbias"] = stats.tile([128, NTJ], F32, name="mdbias", tag="mdbias")
                nc.vector.tensor_scalar_add(s["mdbias"], mdT2, negmd8_b[:, 0:1])

            # ---------- P1 (one half-row of WT per jt step) ----------
            def p1h_jt(k, h, jt):
                s = st[k]
                if jt == 0:
                    s[f"WT{h}"] = wtp.tile(
                        [128, NTJ, HI], BF16, name=f"WT{h}", tag=f"WT{h}"
                    )
                    s[f"macc{h}"] = stats.tile(
                        [128, HI], BF16, name=f"macc{h}", tag=f"macc{h}", bufs=1
                    )
                WT = s[f"WT{h}"]
                psa = ps_att.tile([128, 2, 512], F32, tag="att")
                if cross_fp8:
                    for g in range(2):
                        o = h * HI + g * 512
                        nc.tensor.matmul(
                            psa[:, g, :],
                            lhsT=s["memT8"][:, :, jt * 128 : (jt + 1) * 128],
                            rhs=s["insT8"][:, :, o : o + 512],
                            start=True, stop=True,
                            perf_mode=mybir.MatmulPerfMode.DoubleRow,
                        )
                else:
                    for g in range(2):
                        o = h * HI + g * 512
                        for c in range(KD):
                            nc.tensor.matmul(
                                psa[:, g, :],
                                lhsT=s["memT8"][:, c, jt * 128 : (jt + 1) * 128],
                                rhs=s["insT8"][:, c, o : o + 512],
                                start=(c == 0), stop=(c == KD - 1),
                            )
                nc.scalar.activation(
                    WT[:, jt, :],
                    psa.rearrange("p h x -> p (h x)"),
                    AF.Exp,
                    bias=s["mdbias"][:, jt : jt + 1],
                    scale=1.0 / FP8_SCALE,
                )
                if jt == 0:
                    nc.vector.tensor_copy(s[f"macc{h}"], WT[:, 0, :])
                else:
                    nc.vector.tensor_max(s[f"macc{h}"], s[f"macc{h}"], WT[:, jt, :])

            def m0_h(k, h):
                s = st[k]
                if h == 0:
                    s["m0"] = stats.tile([128, NTI], F32, name="m0", tag="m0")
                for itl in range(NTH):
                    pst = ps_tp.tile([128, 128], BF16, tag="tp", bufs=1)
                    nc.tensor.transpose(
                        pst, s[f"macc{h}"][:, itl * 128 : (itl + 1) * 128], ident_bf
                    )
                    nc.vector.reduce_max(
                        s["m0"][:, h * NTH + itl : h * NTH + itl + 1],
                        pst, axis=mybir.AxisListType.X,
                    )

            def p15(k):
                s = st[k]
                mx2_p = stats.tile([128, 1], F32, tag="mx2_p")
                nc.vector.reduce_max(mx2_p, s["idT"], axis=mybir.AxisListType.X)
                ps_r2 = ps_tp.tile([1, 128], F32, tag="tp", bufs=1)
                nc.tensor.transpose(ps_r2, mx2_p, ident_f32)
                mx2_s = stats.tile([1, 1], F32, tag="mx2_s")
                nc.vector.reduce_max(mx2_s, ps_r2, axis=mybir.AxisListType.X)
                negmx2 = stats.tile([1, 1], F32, tag="negmx2")
                nc.vector.tensor_scalar(
                    out=negmx2, in0=mx2_s, scalar1=-1.0, scalar2=None, op0=AL.mult
                )
                ps_b2 = ps_acc.tile([128, 1], F32, tag="acc")
                nc.tensor.matmul(ps_b2, lhsT=ones_row, rhs=negmx2, start=True, stop=True)
                negmx2_b = stats.tile([128, 1], F32, tag="negmx2b")
                nc.vector.tensor_copy(negmx2_b, ps_b2)

                eid = stats.tile([128, NTI], F32, tag="eid")
                nc.scalar.activation(eid, s["idT"], AF.Exp, bias=negmx2_b[:, 0:1], scale=1.0)
                e2 = stats.tile([128, NTI], F32, tag="e2")
                nc.vector.tensor_mul(e2, eid, s["m0"])
                sum2_p = stats.tile([128, 1], F32, tag="sum2_p")
                nc.vector.reduce_sum(sum2_p, e2, axis=mybir.AxisListType.X)
                ps_r3 = ps_tp.tile([1, 128], F32, tag="tp", bufs=1)
                nc.tensor.transpose(ps_r3, sum2_p, ident_f32)
                sum2_s = stats.tile([1, 1], F32, tag="sum2_s")
                nc.vector.reduce_sum(sum2_s, ps_r3, axis=mybir.AxisListType.X)
                r2 = stats.tile([1, 1], F32, tag="r2")
                nc.vector.reciprocal(r2, sum2_s)
                ps_b3 = ps_acc.tile([128, 1], F32, tag="acc")
                nc.tensor.matmul(ps_b3, lhsT=ones_row, rhs=r2, start=True, stop=True)
                r2_b = stats.tile([128, 1], F32, tag="r2b")
                nc.vector.tensor_copy(r2_b, ps_b3)
                w2 = stats.tile([128, NTI], F32, tag="w2")
                nc.vector.tensor_scalar_mul(w2, e2, r2_b[:, 0:1])

                o2T = stats.tile([128, KD], F32, tag="o2T")
                for c in range(KD):
                    ps_o2 = ps_acc.tile([128, 1], F32, tag="acc")
                    for it in range(NTI):
                        nc.tensor.matmul(
                            ps_o2,
                            lhsT=s["in_sb"][:, it, c * 128 : (c + 1) * 128],
                            rhs=w2[:, it : it + 1],
                            start=(it == 0), stop=(it == NTI - 1),
                        )
                    nc.vector.tensor_copy(o2T[:, c : c + 1], ps_o2)
                o2row = stats.tile([1, d], F32, tag="o2row")
                for c in range(KD):
                    ps_r4 = ps_tp.tile([1, 128], F32, tag="tp", bufs=1)
                    nc.tensor.transpose(ps_r4, o2T[:, c : c + 1], ident_f32)
                    nc.vector.tensor_copy(o2row[:, c * 128 : (c + 1) * 128], ps_r4)
                ps_o2b = ps_acc.tile([128, d], F32, tag="acc")
                nc.tensor.matmul(ps_o2b, lhsT=ones_row, rhs=o2row, start=True, stop=True)
                s["out2b"] = stats.tile([128, d], F32, name="out2b", tag="out2b", bufs=1)
                nc.vector.tensor_copy(s["out2b"], ps_o2b)

            # ---------- P2 (per half, interleaved into the next P1) ------
            def p2h_mm(k, h, itl):
                s = st[k]
                if h == 0 and itl == 0:
                    s["o1k"] = work.tile(
                        [128, NTI, d], BF16, name="o1k", tag="o1k", bufs=1
                    )
                    s["psO"] = {}
                psO = ps_acc.tile([128, DE], F32, tag="acc")
                s["psO"][(h, itl)] = psO
                for jt in range(NTJ):
                    nc.tensor.matmul(
                        psO,
                        lhsT=s[f"WT{h}"][:, jt, itl * 128 : (itl + 1) * 128],
                        rhs=s["mem_ext"][:, jt, :],
                        start=(jt == 0), stop=(jt == NTJ - 1),
                    )

            def p2h_fin(k, h, itl):
                s = st[k]
                b = batches[k]
                it = h * NTH + itl
                out_v = out[b].rearrange("(t p) f -> p t f", p=128)
                psO = s["psO"].pop((h, itl))
                rden = stats.tile([128, 1], F32, tag="rden")
                nc.vector.reciprocal(rden, psO[:, d : d + 1])
                o1f = outp.tile([128, d], F32, tag="o1f")
                nc.vector.tensor_scalar_mul(o1f, psO[:, 0:d], rden[:, 0:1])
                nc.gpsimd.tensor_copy(s["o1k"][:, it, :], o1f)
                io = outp.tile([128, d], F32, tag="io")
                nc.gpsimd.tensor_mul(io, s["in_sb"][:, it, :], o1f)
                nc.sync.dma_start(out=out_v[:, it, 0:d], in_=s["in_sb"][:, it, :])
                nc.sync.dma_start(out=out_v[:, it, d : 2 * d], in_=o1f)
                nc.sync.dma_start(out=out_v[:, it, 2 * d : 3 * d], in_=io)

            def oo_fin(k, it):
                s = st[k]
                b = batches[k]
                out_v = out[b].rearrange("(t p) f -> p t f", p=128)
                oo = outp.tile([128, d], F32, tag="oo")
                nc.gpsimd.tensor_mul(oo, s["out2b"], s["o1k"][:, it, :])
                nc.sync.dma_start(out=out_v[:, it, 3 * d : 4 * d], in_=oo)
                if it == NTI - 1:
                    del st[k]

            # ---------- schedule ----------
            # host-chunk streams consumed one per (k, h, jt) slot
            def host_chunks(k, h):
                """P0/P2/oo work hosted in the 16 slots of phase (k, h)."""
                ch = [[] for _ in range(NTJ)]
                if h == 0:
                    # P2B(k-1), deferred oo(k-1), casts(k+1), mdid(k+1) 0-6
                    if k - 1 >= 0:
                        for e in range(NTH):
                            ch[2 * e].append(lambda kk=k - 1, ee=e: p2h_mm(kk, 1, ee))
                            ch[2 * e + 1].append(lambda kk=k - 1, ee=e: p2h_fin(kk, 1, ee))
                        for i in range(NTI):
                            ch[i if i < NTJ else NTJ - 1].append(
                                lambda kk=k - 1, ii=i: oo_fin(kk, ii)
                            )
                    if k + 1 < K:
                        ch[8].append(lambda kk=k + 1: c_casts(kk))
                        for jr in range(7):
                            ch[9 + jr].append(lambda kk=k + 1, j=jr: c_mdid(kk, j))
                else:
                    # P2A(k), mdid(k+1) last, transposes(k+1), loads(k+2)
                    for e in range(NTH):
                        ch[2 * e].append(lambda ee=e: p2h_mm(k, 0, ee))
                        ch[2 * e + 1].append(lambda ee=e: p2h_fin(k, 0, ee))
                    if k + 1 < K:
                        ch[0].append(lambda kk=k + 1: c_mdid(kk, 7))
                        for r in range(8):
                            ch[1 + r].append(lambda kk=k + 1, j0=2 * r: c_tp_mem(kk, j0))
                        for r in range(7):
                            ch[9 + r].append(lambda kk=k + 1, i0=2 * r: c_tp_in(kk, i0))
                    if k + 2 < K:
                        ch[0].append(lambda kk=k + 2: c_loads(kk))
                return ch

            def late_chunks(k):
                """transpose rounds that spill past the (k,1) slots."""
                ops = []
                if k + 1 < K:
                    for r in range(7, 8):
                        ops.append(lambda kk=k + 1, i0=2 * r: c_tp_in(kk, i0))
                return ops

            # fused prologue.  Tile resolves dataflow in EMISSION order, so
            # everything the P1A(0) exps/MMs read must be emitted first:
            # the full mdbias chain (mdid + tail), and for each cross MM its
            # memT8 block / the whole half-A insT8 span.
            c_loads(0)
            for jr in range(8):
                c_mdid(0, jr, which="md")
            p0_tail(0)
            pre0 = [[] for _ in range(NTJ)]
            post0 = [[] for _ in range(NTJ)]
            pre0[0].append(lambda: c_casts(0, half=0))
            for r in range(4):
                pre0[0].append(lambda i0=2 * r: c_tp_in(0, i0))
            pre0[2].append(lambda: c_casts(0, half=1))
            for r in range(8):
                pre0[r].append(lambda j0=2 * r: c_tp_mem(0, j0))
            for r in range(4, 8):
                post0[4 + r].append(lambda i0=2 * r: c_tp_in(0, i0))
            for jr in range(8):
                post0[4 + jr].append(lambda j=jr: c_mdid(0, j, which="id"))
            if K > 1:
                post0[4].append(lambda: c_loads(1))

            for k in range(K):
                for h in (0, 1):
                    ch = host_chunks(k, h)
                    pre = pre0 if (k == 0 and h == 0) else [[]] * NTJ
                    post = post0 if (k == 0 and h == 0) else None
                    for jt in range(NTJ):
                        for fn in pre[jt]:
                            fn()
                        p1h_jt(k, h, jt)
                        for fn in ch[jt]:
                            fn()
                        if post is not None:
                            for fn in post[jt]:
                                fn()
                    m0_h(k, h)
                    if h == 1:
                        for fn in late_chunks(k):
                            fn()
                        if k + 1 < K:
                            p0_tail(k + 1)
                        p15(k)
            # drain: P2B and oo of the last batch, interleaved
            kl = K - 1
            for e in range(NTH):
                p2h_mm(kl, 1, e)
                p2h_fin(kl, 1, e)
                oo_fin(kl, e)
                oo_fin(kl, NTH + e)

    nc.compile()
    return nc


_CACHE = {}


def _get_nc():
    if "nc" not in _CACHE:
        _CACHE["nc"] = build_program()
    return _CACHE["nc"]


def kernel(input, memory, mask, w_input, w_memory, dot_scale):
    from concourse.bass_utils import run_bass_kernel_spmd

    nc = _get_nc()
    input = np.ascontiguousarray(np.asarray(input, dtype=np.float32))
    memory = np.ascontiguousarray(np.asarray(memory, dtype=np.float32))
    mask = np.ascontiguousarray(np.asarray(mask, dtype=np.float32))
    w_input = np.ascontiguousarray(np.asarray(w_input, dtype=np.float32))
    w_memory = np.ascontiguousarray(np.asarray(w_memory, dtype=np.float32))
    dot_scale = np.ascontiguousarray(np.asarray(dot_scale, dtype=np.float32))

    in_maps = []
    for c in range(N_CORES):
        sl = slice(c * B_PC, (c + 1) * B_PC)
        in_maps.append(
            {
                "input": input[sl],
                "memory": memory[sl],
                "mask": mask[sl],
                "w_input": w_input,
                "w_memory": w_memory,
                "dot_scale": dot_scale,
            }
        )
    res = run_bass_kernel_spmd(nc, in_maps, core_ids=list(range(N_CORES)))
    return np.concatenate([r["output"] for r in res.results], axis=0)

